# revision 1
# baseline (speedup 1.0000x reference)
"""CRF log-loss kernel for TRN2, data-parallel over batch on 8 NeuronCores.

Algorithm (per core, 128 examples):
  Forward algorithm in the exp domain:
      u_{s+1}[j,b] = (sum_k exp(trans[j,k] + LN_SCALE) * u_s[k,b]) * exp(feat[b,s,j] + beta)
  One 64x65 stationary-weight matmul (65th row = column sums, used for
  renormalization feedback) + one vector multiply per step. Periodic
  per-example renormalization is applied as a per-partition bias inside the
  bulk exp(feats) on the scalar engine, with an exponent-extract rough log
  on the vector engine as feedback; exact log accounting happens once at
  the end. Gold-path score via iota==tag masks (emission) and gpsimd
  ap_gather from a replicated transition table (transition score).
"""
import numpy as np
import ml_dtypes
from contextlib import ExitStack

import concourse.bass as bass
import concourse.bacc as bacc
import concourse.tile as tile
import concourse.mybir as mybir
from concourse.bass_utils import run_bass_kernel_spmd

bf16 = ml_dtypes.bfloat16
f32 = mybir.dt.float32
bf16d = mybir.dt.bfloat16
i16 = mybir.dt.int16
u16 = mybir.dt.uint16
i32 = mybir.dt.int32

B, S, T = 1024, 512, 64
NC = 8
BC = B // NC            # 128 examples per core
CHUNK = 8               # steps per renorm/exp chunk
NCH = S // CHUNK        # 64 chunks
LAG = 2                 # controller application lag (in chunks)
LN_SCALE = -4.7         # mean drift folded into PA
LN2 = float(np.log(2.0))

AF = mybir.ActivationFunctionType
ALU = mybir.AluOpType
AXX = mybir.AxisListType.X


def _build_program():
    nc = bacc.Bacc("TRN2", target_bir_lowering=False, debug=False, num_devices=NC)

    feats_d = nc.dram_tensor("feats", [BC, S, T], f32, kind="ExternalInput")
    u0_d = nc.dram_tensor("u0", [T, BC], bf16d, kind="ExternalInput")
    pa_d = nc.dram_tensor("pa", [T, T + 1], bf16d, kind="ExternalInput")
    pfin_d = nc.dram_tensor("pfin", [T, 1], bf16d, kind="ExternalInput")
    hmask_d = nc.dram_tensor("hmask", [BC, S, T], bf16d, kind="ExternalInput")
    startw_d = nc.dram_tensor("startw", [BC, T], f32, kind="ExternalInput")
    transrep_d = nc.dram_tensor("transrep", [BC, T * T], f32, kind="ExternalInput")
    pairsw_d = nc.dram_tensor("pairsw", [BC, 16 * 32], u16, kind="ExternalInput")
    m16_d = nc.dram_tensor("m16", [BC, 16], bf16d, kind="ExternalInput")
    out_d = nc.dram_tensor("out", [BC, 1], f32, kind="ExternalOutput")

    with tile.TileContext(nc) as tc, ExitStack() as ctx:
        cpool = ctx.enter_context(tc.tile_pool(name="const", bufs=1))
        fpool = ctx.enter_context(tc.tile_pool(name="feats", bufs=3))
        epool = ctx.enter_context(tc.tile_pool(name="ech", bufs=3))
        etpool = ctx.enter_context(tc.tile_pool(name="ett", bufs=8))
        upool = ctx.enter_context(tc.tile_pool(name="u", bufs=4))
        pspool = ctx.enter_context(tc.tile_pool(name="ps", bufs=4, space="PSUM"))
        ps2pool = ctx.enter_context(tc.tile_pool(name="ps2", bufs=1, space="PSUM"))
        bhpool = ctx.enter_context(tc.tile_pool(name="bh", bufs=4))
        mpool = ctx.enter_context(tc.tile_pool(name="mask", bufs=2))
        scpool = ctx.enter_context(tc.tile_pool(name="scratch", bufs=2))
        gpool = ctx.enter_context(tc.tile_pool(name="gather", bufs=2))

        # ---- constants into SBUF ----
        pa_s = cpool.tile([T, T + 1], bf16d)
        nc.sync.dma_start(pa_s[:, :], pa_d[:, :])
        pfin_s = cpool.tile([T, 1], bf16d)
        nc.sync.dma_start(pfin_s[:, :], pfin_d[:, :])
        h0_s = cpool.tile([BC, T], bf16d)
        nc.sync.dma_start(h0_s[:, :], hmask_d[:, 0, :])
        hL_s = cpool.tile([BC, T], bf16d)
        nc.sync.dma_start(hL_s[:, :], hmask_d[:, S - 1, :])
        startw_s = cpool.tile([BC, T], f32)
        nc.sync.dma_start(startw_s[:, :], startw_d[:, :])
        transrep_s = cpool.tile([BC, T * T], f32)
        nc.sync.dma_start(transrep_s[:, :], transrep_d[:, :])
        pairsw_s = cpool.tile([BC, 16 * 32], u16)
        nc.sync.dma_start(pairsw_s[:, :], pairsw_d[:, :])
        m16_s = cpool.tile([BC, 16], bf16d)
        nc.sync.dma_start(m16_s[:, :], m16_d[:, :])

        id1 = cpool.tile([1, 1], f32)
        nc.vector.memset(id1[:, :], 1.0)
        zcol = cpool.tile([BC, 1], f32)
        nc.vector.memset(zcol[:, :], 0.0)

        zrow = cpool.tile([1, BC], f32)
        nc.vector.memset(zrow[:, :], 0.0)

        # emission partial sums, one column per chunk
        parts = cpool.tile([BC, NCH], f32)
        # gathered-transition reduction columns, one per gather call
        rt16 = cpool.tile([BC, 16], f32)

        ucur = upool.tile([T, BC], bf16d)
        nc.sync.dma_start(ucur[:, :], u0_d[:, :])

        # ---- gold: transition-score gathers (independent of the chain) ----
        # priming copies: pool instructions encode at most ONE sync wait, so
        # make gpsimd observe each input tile one at a time up front
        pr1 = scpool.tile([BC, 1], f32)
        nc.gpsimd.tensor_copy(pr1[:, :], transrep_s[:, 0:1])
        pr2 = scpool.tile([BC, 1], u16)
        nc.gpsimd.tensor_copy(pr2[:, :], pairsw_s[:, 0:1])
        for i in range(16):
            gout = gpool.tile([BC, 512], f32)
            nc.gpsimd.indirect_copy(
                gout[:, :].unsqueeze(-1),
                transrep_s[:, :],
                pairsw_s[:, i * 32:(i + 1) * 32],
                i_know_ap_gather_is_preferred=True,
            )
            nc.vector.tensor_reduce(rt16[:, i:i + 1], gout[:, 0:511], axis=AXX, op=ALU.add)

        # ---- main loop ----
        biases = []  # per-chunk ACT bias tiles
        bprev = zrow
        grow = zrow
        for t in range(NCH):
            fch = fpool.tile([BC, CHUNK, T], f32)
            nc.sync.dma_start(fch[:, :, :], feats_d[:, t * CHUNK:(t + 1) * CHUNK, :])

            bias_ap = zcol[:, :] if t < LAG else biases[t - LAG]
            ech = epool.tile([BC, CHUNK * T], bf16d)
            nc.scalar.activation(ech[:, :], fch[:, :, :].rearrange("p a b -> p (a b)"),
                                 AF.Exp, bias=bias_ap, scale=1.0)

            # transpose E to [(s,j), b] in pairs of steps via DMA xbar
            etts = []
            for p in range(CHUNK // 2):
                ett = etpool.tile([2 * T, BC], bf16d)
                nc.sync.dma_start_transpose(ett[:, :], ech[:, p * 2 * T:(p + 1) * 2 * T])
                etts.append(ett)

            # gold emission: fused (feats * onehot) with free-dim accumulate
            hch = mpool.tile([BC, CHUNK, T], bf16d)
            nc.sync.dma_start(hch[:, :, :], hmask_d[:, t * CHUNK:(t + 1) * CHUNK, :])
            sc = scpool.tile([BC, CHUNK * T], f32)
            nc.vector.scalar_tensor_tensor(
                sc[:, :], fch[:, :, :].rearrange("p a b -> p (a b)"), 1.0,
                hch[:, :, :].rearrange("p a b -> p (a b)"),
                op0=ALU.mult, op1=ALU.mult,
                accum_out=parts[:, t:t + 1])

            # chain steps
            pt = None
            for sl in range(CHUNK):
                pt = pspool.tile([T + 1, BC], f32)
                nc.tensor.matmul(pt[:, :], pa_s[:, :], ucur[:, :], start=True, stop=True)
                unext = upool.tile([T, BC], bf16d)
                ett = etts[sl // 2]
                h = (sl % 2) * T
                nc.vector.tensor_tensor(unext[:, :], pt[0:T, :], ett[h:h + T, :], ALU.mult)
                ucur = unext

            # renorm controller from the last step's column sums
            if t + LAG < NCH:
                eint = scpool.tile([1, BC], i32)
                nc.vector.tensor_scalar(eint[:, :], pt[T:T + 1, :].bitcast(i32),
                                        23, None, op0=ALU.logical_shift_right)
                lam2 = scpool.tile([1, BC], f32)
                nc.vector.tensor_scalar(lam2[:, :], eint[:, :],
                                        127, -LN2 / CHUNK,
                                        op0=ALU.subtract, op1=ALU.mult)
                brow = bhpool.tile([1, BC], f32)
                nc.vector.tensor_sub(brow[:, :], lam2[:, :], bprev[:, :])
                bprev = brow
                gnew = bhpool.tile([1, BC], f32)
                nc.vector.scalar_tensor_tensor(
                    gnew[:, :], brow[:, :], float(CHUNK),
                    grow[:, :], op0=ALU.mult, op1=ALU.add)
                grow = gnew
                pbt = ps2pool.tile([BC, 1], f32)
                nc.tensor.transpose(pbt[:, :], brow[:, :], id1[:, :])
                bh = bhpool.tile([BC, 1], f32)
                nc.vector.tensor_copy(bh[:, :], pbt[:, :])
                biases.append(bh[:, :])

        # ---- finalization ----
        ptf = ps2pool.tile([1, BC], f32)
        nc.tensor.matmul(ptf[:, :], pfin_s[:, :], ucur[:, :], start=True, stop=True)

        lamf = scpool.tile([1, BC], f32)
        nc.scalar.activation(lamf[:, :], ptf[:, :], AF.Ln)
        fwdr = scpool.tile([1, BC], f32)
        nc.vector.tensor_sub(fwdr[:, :], lamf[:, :], grow[:, :])
        pfw = ps2pool.tile([BC, 1], f32)
        nc.tensor.transpose(pfw[:, :], fwdr[:, :], id1[:, :])

        # gold assembly
        emitsum = scpool.tile([BC, 1], f32)
        nc.vector.tensor_reduce(emitsum[:, :], parts[:, :], axis=AXX, op=ALU.add)
        sc16 = scpool.tile([BC, 16], f32)
        goldtr = scpool.tile([BC, 1], f32)
        nc.vector.scalar_tensor_tensor(
            sc16[:, :], rt16[:, :], 1.0, m16_s[:, :],
            op0=ALU.mult, op1=ALU.mult, accum_out=goldtr[:, :])

        sc0 = scpool.tile([BC, T], f32)
        s0col = scpool.tile([BC, 1], f32)
        nc.vector.scalar_tensor_tensor(
            sc0[:, :], startw_s[:, :], 1.0, h0_s[:, :],
            op0=ALU.mult, op1=ALU.mult, accum_out=s0col[:, :])
        scL = scpool.tile([BC, T], f32)
        sLcol = scpool.tile([BC, 1], f32)
        nc.vector.scalar_tensor_tensor(
            scL[:, :], startw_s[:, :], 1.0, hL_s[:, :],
            op0=ALU.mult, op1=ALU.mult, accum_out=sLcol[:, :])

        g1 = scpool.tile([BC, 1], f32)
        nc.vector.tensor_add(g1[:, :], s0col[:, :], sLcol[:, :])
        g2 = scpool.tile([BC, 1], f32)
        nc.vector.tensor_add(g2[:, :], g1[:, :], emitsum[:, :])
        g3 = scpool.tile([BC, 1], f32)
        nc.vector.tensor_add(g3[:, :], g2[:, :], goldtr[:, :])

        l0 = scpool.tile([BC, 1], f32)
        nc.vector.tensor_sub(l0[:, :], pfw[:, :], g3[:, :])
        lout = scpool.tile([BC, 1], f32)
        nc.vector.tensor_scalar(lout[:, :], l0[:, :], -S * LN_SCALE, None, op0=ALU.add)
        nc.sync.dma_start(out_d[:, :], lout[:, :])

    nc.compile()
    return nc


def _host_constants(transitions, start_tag, tags):
    """Small host-side constant tensors (index plumbing + exp of the tiny
    transition matrix); tags comes in as [B, S] int."""
    pa = np.zeros((T, T + 1), dtype=np.float32)
    pa[:, :T] = np.exp(transitions.T + LN_SCALE)
    pa[:, T] = 1.0
    pa = pa.astype(bf16)
    pfin = np.exp(transitions[T - 1, :]).astype(bf16).reshape(T, 1)
    u0 = np.tile(np.exp(start_tag).astype(np.float32)[:, None], (1, BC)).astype(bf16)
    startw = np.tile(start_tag.astype(np.float32)[None, :], (BC, 1))
    transrep = np.tile(transitions.astype(np.float32).reshape(1, T * T), (BC, 1))
    m16 = np.zeros((BC, 16), dtype=bf16)
    for p in range(BC):
        m16[p, p % 16] = 1

    # one-hot of the gold tags, bf16 (streamed next to feats for the
    # emission-score multiply-accumulate)
    tags_i = tags.astype(np.int64)
    hmask = (tags_i[:, :, None] == np.arange(T)[None, None, :]).astype(bf16)

    # wrapped pair indices for the indirect_copy gathers: instr i,
    # 16-partition group g handles example b = g*16 + i; unwrapped order is
    # (c*16 + p).
    pairs = np.zeros((B, 512), dtype=np.uint16)
    pairs[:, :511] = (tags_i[:, :511] * T + tags_i[:, 1:512]).astype(np.uint16)
    gi, pi, ci = np.meshgrid(np.arange(8), np.arange(16), np.arange(32),
                             indexing="ij")
    pairsw = np.zeros((NC, BC, 16 * 32), dtype=np.uint16)
    for c in range(NC):
        pc = pairs[c * BC:(c + 1) * BC]
        for i in range(16):
            b = gi * 16 + i
            s = ci * 16 + pi
            pairsw[c, (16 * gi + pi).reshape(-1), (i * 32 + ci).reshape(-1)] =                 pc[b.reshape(-1), s.reshape(-1)]
    return pa, pfin, u0, startw, transrep, m16, pairsw, hmask


_NC_CACHE = {}


def _get_program():
    if "nc" not in _NC_CACHE:
        _NC_CACHE["nc"] = _build_program()
    return _NC_CACHE["nc"]


def kernel(feats, transitions, start_tag, tags, mask_x, len_seq):
    feats = np.asarray(feats, dtype=np.float32)
    transitions = np.asarray(transitions, dtype=np.float32)
    start_tag = np.asarray(start_tag, dtype=np.float32)
    tags_np = np.asarray(tags)
    out_dtype = np.float32

    pa, pfin, u0, startw, transrep, m16, pairsw, hmask = \
        _host_constants(transitions, start_tag, tags_np)

    in_maps = []
    for c in range(NC):
        sl = slice(c * BC, (c + 1) * BC)
        in_maps.append({
            "feats": np.ascontiguousarray(feats[sl]),
            "hmask": np.ascontiguousarray(hmask[sl]),
            "u0": u0, "pa": pa, "pfin": pfin, "startw": startw,
            "transrep": transrep, "pairsw": pairsw[c], "m16": m16,
        })

    nc = _get_program()
    res = run_bass_kernel_spmd(nc, in_maps, list(range(NC)))
    out = np.concatenate([res.results[i]["out"][:, 0] for i in range(NC)])
    return out.astype(out_dtype)



# revision 5
# speedup vs baseline: 5.0139x; 5.0139x over previous
"""CRF log-loss kernel for TRN2 — forward/backward split across core pairs.

Device mapping (8 NeuronCores = 4 pairs, 256 examples per pair):
  Even core of pair p: forward recursion  U_{s+1} = e_s ∘ (A' U_s), steps 0..255.
  Odd  core of pair p: backward recursion G_{i+1} = e_{510-i} ∘ (A'^T G_i),
      seeded with G_0 = e_511 ∘ exp(trans[stop,:]) and closed with one extra
      A'^T application fed e=exp(0)=1 (a zeros step appended to its feats
      stream) — so both cores run the IDENTICAL program, differing only in
      their input tensors (stationary matrix, init state, feats slice).
  Host combines: fwd = ln(Σ_k U_256[k,b] · G_256[k,b]) - 512·LN_SCALE.

Per step the device does exactly one [128,128] bf16 matmul (stationary =
block-diag packed transition matrix, two 64-tag halves carrying 128 examples
each) and one [128,128] tensor_tensor multiply with the pre-exponentiated
feats slice. Feats are transposed to [s, h, tag, col] layout on the host, so
no on-device transposes exist; the drift constant LN_SCALE (estimated from
the data at run time) is folded into the transition matrix, so no renorm
controller exists. The gold (numerator) path is pure index plumbing and is
computed on the host.
"""
import numpy as np
import ml_dtypes
from contextlib import ExitStack

import concourse.bass as bass
import concourse.bacc as bacc
import concourse.tile as tile
import concourse.mybir as mybir
from concourse.bass_utils import run_bass_kernel_spmd

bf16 = ml_dtypes.bfloat16
f32 = mybir.dt.float32
bf16d = mybir.dt.bfloat16

B, S, T = 1024, 512, 64
NC = 8
NPAIR = NC // 2
EPC = B // NPAIR        # 256 examples per core pair
HALF = S // 2           # 256 steps per core
CH = 8                  # steps per DMA/exp chunk
NCH = HALF // CH

AF = mybir.ActivationFunctionType
ALU = mybir.AluOpType


def _build_program():
    nc = bacc.Bacc("TRN2", target_bir_lowering=False, debug=False, num_devices=NC)

    featsT_d = nc.dram_tensor("featsT", [HALF, 2, T, 128], f32, kind="ExternalInput")
    pak_d = nc.dram_tensor("pak", [128, 128], bf16d, kind="ExternalInput")
    u0_d = nc.dram_tensor("u0", [128, 128], bf16d, kind="ExternalInput")
    uout_d = nc.dram_tensor("uout", [128, 128], bf16d, kind="ExternalOutput")

    with tile.TileContext(nc) as tc, ExitStack() as ctx:
        cpool = ctx.enter_context(tc.tile_pool(name="const", bufs=1))
        fpool = ctx.enter_context(tc.tile_pool(name="feats", bufs=3))
        epool = ctx.enter_context(tc.tile_pool(name="ech", bufs=3))
        upool = ctx.enter_context(tc.tile_pool(name="u", bufs=4))
        pspool = ctx.enter_context(tc.tile_pool(name="ps", bufs=4, space="PSUM"))

        pak_s = cpool.tile([128, 128], bf16d)
        nc.sync.dma_start(pak_s[:, :], pak_d[:, :])
        ucur = upool.tile([128, 128], bf16d)
        nc.sync.dma_start(ucur[:, :], u0_d[:, :])

        for t in range(NCH):
            fch = fpool.tile([128, CH, 128], f32)
            nc.sync.dma_start(
                fch[:, :, :],
                featsT_d[t * CH:(t + 1) * CH, :, :, :].rearrange(
                    "s h k c -> (h k) s c"),
            )
            ech = epool.tile([128, CH, 128], bf16d)
            nc.scalar.activation(
                ech[:, :, :].rearrange("p a b -> p (a b)"),
                fch[:, :, :].rearrange("p a b -> p (a b)"),
                AF.Exp,
            )
            for i in range(CH):
                pt = pspool.tile([128, 128], f32)
                nc.tensor.matmul(pt[:, :], pak_s[:, :], ucur[:, :],
                                 start=True, stop=True)
                unext = upool.tile([128, 128], bf16d)
                nc.vector.tensor_tensor(unext[:, :], pt[:, :], ech[:, i, :],
                                        ALU.mult)
                ucur = unext

        nc.sync.dma_start(uout_d[:, :], ucur[:, :])

    nc.compile()
    return nc


def _estimate_ln_scale(feats, transitions, start_tag):
    """Mean per-step log growth of the forward recursion, measured on a few
    examples/steps so the folded scale keeps the exp-domain state centered."""
    n_ex, n_st = 8, 64
    A = np.exp(transitions.astype(np.float64))
    score = np.tile(start_tag.astype(np.float64)[None, :], (n_ex, 1))
    f = feats[:n_ex, :n_st, :].astype(np.float64)
    lam0 = lamN = None
    for s in range(n_st):
        m = score.max(1, keepdims=True)
        score = np.log(np.exp(score - m) @ A.T) + m + f[:, s, :]
        lse = np.log(np.exp(score - score.max(1, keepdims=True)).sum(1)) \
            + score.max(1)
        if s == 0:
            lam0 = lse
        lamN = lse
    return -float((lamN - lam0).mean() / (n_st - 1))


def _host_inputs(feats, transitions, start_tag):
    """Per-core input tensors. Returns (in_maps, ln_scale)."""
    ln_scale = _estimate_ln_scale(feats, transitions, start_tag)
    Ap = np.exp(transitions.astype(np.float64) + ln_scale).astype(np.float32)

    def pack_blockdiag(m):
        out = np.zeros((128, 128), dtype=np.float32)
        out[:T, :T] = m
        out[T:, T:] = m
        return out.astype(bf16)

    # fw matmul: out[j] = Σ_k A'[j,k] u[k]  → lhsT[k, j] = A'[j, k] = Ap.T
    pak_fw = pack_blockdiag(Ap.T)
    # bw matmul: out[k] = Σ_j A'[j,k] g[j]  → lhsT[j, k] = A'[j, k] = Ap
    pak_bw = pack_blockdiag(Ap)

    # u0_fw[(h*64+k), c] = exp(start[k])
    u0_fw = np.tile(np.exp(start_tag.astype(np.float32)), 2)[:, None] \
        * np.ones((1, 128), np.float32)
    u0_fw = u0_fw.astype(bf16)

    R = np.exp(transitions[T - 1, :].astype(np.float32))  # stop row, no scale

    in_maps = []
    for p in range(NPAIR):
        exsl = slice(p * EPC, (p + 1) * EPC)
        x = feats[exsl]                      # [256, S, T] f32
        # fw core: steps 0..255; F[s, h, k, c] = x[128h + c, s, k]
        xf = x[:, :HALF, :].reshape(2, 128, HALF, T).transpose(2, 0, 3, 1)
        featsT_fw = np.ascontiguousarray(xf, dtype=np.float32)
        # bw core: steps 510..256 reversed, then one zeros step
        xb = x[:, 256:511, :][:, ::-1, :]    # [256, 255, T] = e_510 .. e_256
        xb = xb.reshape(2, 128, 255, T).transpose(2, 0, 3, 1)
        featsT_bw = np.zeros((HALF, 2, T, 128), dtype=np.float32)
        featsT_bw[:255] = xb
        # bw init: G0[(h*64+j), c] = exp(x[128h+c, 511, j]) * R[j]
        e511 = np.exp(x[:, 511, :].astype(np.float32)) * R[None, :]  # [256, T]
        u0_bw = e511.reshape(2, 128, T).transpose(0, 2, 1).reshape(128, 128)
        u0_bw = u0_bw.astype(bf16)

        in_maps.append({"featsT": featsT_fw, "pak": pak_fw, "u0": u0_fw})
        in_maps.append({"featsT": featsT_bw, "pak": pak_bw, "u0": u0_bw})
    return in_maps, ln_scale


def _host_gold(feats, transitions, start_tag, tags):
    tags_i = tags.astype(np.int64)
    emit = np.take_along_axis(feats, tags_i[:, :, None], axis=2)[:, :, 0]
    trans_sc = transitions[tags_i[:, :-1], tags_i[:, 1:]]
    gold = (start_tag[tags_i[:, 0]] + emit.sum(1, dtype=np.float64)
            + trans_sc.sum(1, dtype=np.float64) + start_tag[tags_i[:, -1]])
    return gold


_NC_CACHE = {}


def _get_program():
    if "nc" not in _NC_CACHE:
        _NC_CACHE["nc"] = _build_program()
    return _NC_CACHE["nc"]


def kernel(feats, transitions, start_tag, tags, mask_x, len_seq):
    feats = np.asarray(feats, dtype=np.float32)
    transitions = np.asarray(transitions, dtype=np.float32)
    start_tag = np.asarray(start_tag, dtype=np.float32)
    tags_np = np.asarray(tags)

    in_maps, ln_scale = _host_inputs(feats, transitions, start_tag)
    nc = _get_program()
    res = run_bass_kernel_spmd(nc, in_maps, list(range(NC)))

    fwd = np.empty(B, dtype=np.float64)
    for p in range(NPAIR):
        U = np.asarray(res.results[2 * p]["uout"]).astype(np.float64)
        G = np.asarray(res.results[2 * p + 1]["uout"]).astype(np.float64)
        # row (h*64+k), col c  ->  example 128h+c (pair-local), tag k
        d = (U.reshape(2, T, 128) * G.reshape(2, T, 128)).sum(1)  # [2, 128]
        fwd[p * EPC:(p + 1) * EPC] = np.log(d.reshape(EPC)) - S * ln_scale

    gold = _host_gold(feats, transitions, start_tag, tags_np)
    return (fwd - gold).astype(np.float32)


# revision 7
# speedup vs baseline: 8.0453x; 1.6046x over previous
"""CRF log-loss kernel for TRN2 — 5-way sequence split with rank-1 junctions.

The S=512-step forward recursion U_{s+1} = e_s ∘ (A' U_s) is split into 5
segments. Products of positive matrices contract to rank-1 exponentially
fast (measured sigma2/sigma1 ~ 1e-16 over 100 steps here), so each middle
segment's transfer matrix M_i is represented by a forward probe f_i = M_i z
and a backward probe g_i = M_i^T w:  M_i ≈ f_i g_i^T / (w^T M_i z), exact
for rank-1. The full path value

  r^T M5 M4 M3 M2 M1 u0  ≈  (b5·f4)(g4·f3)(g3·f2)(g2·f1) / (c4 c3 c2)

is assembled on the host from per-example 64-vector dots. 8 cores run the
IDENTICAL 103-step program (one [128,128]x[128,512] bf16 matmul + one
[128,512] multiply per step, two independently-semaphored 256-column
chains to hide latency), differing only in inputs: stationary matrix
(A'^T-pack for forward runs, A'-pack for backward), init state, and feats
stream (per-step [tag, example] slices prepared on the host in bf16;
backward streams are time-reversed, closed with a zeros step exp(0)=1 that
realizes the trailing bare A'^T, and short segments are padded with leading
zeros steps that merely warm the arbitrary probe seed). The drift constant
LN_SCALE (estimated from the data at run time) is folded into A', so the
exp-domain state needs no renormalization. The gold path is host-side
index plumbing.
"""
import numpy as np
import ml_dtypes
from contextlib import ExitStack

import concourse.bass as bass
import concourse.bacc as bacc
import concourse.tile as tile
import concourse.mybir as mybir
from concourse.bass_utils import run_bass_kernel_spmd

bf16 = ml_dtypes.bfloat16
f32 = mybir.dt.float32
bf16d = mybir.dt.bfloat16

B, S, T = 1024, 512, 64
NC = 8
L = 103                  # program steps per core
COLS = 512               # matmul columns (1024 examples packed 2-per-column)
HC = COLS // 2           # columns per chain
CHUNKS = [8] * 12 + [7]  # DMA/exp chunk sizes, sum = 103

AF = mybir.ActivationFunctionType
ALU = mybir.AluOpType

# segment boundaries [0,103),[103,206),[206,309),[309,409),[409,512)
SEG = [0, 103, 206, 309, 409, 512]


def _build_program():
    nc = bacc.Bacc("TRN2", target_bir_lowering=False, debug=False, num_devices=NC)

    featsT_d = nc.dram_tensor("featsT", [L, 2, T, COLS], bf16d, kind="ExternalInput")
    pak_d = nc.dram_tensor("pak", [128, 128], bf16d, kind="ExternalInput")
    u0_d = nc.dram_tensor("u0", [128, COLS], bf16d, kind="ExternalInput")
    uout_d = nc.dram_tensor("uout", [128, COLS], bf16d, kind="ExternalOutput")

    with tile.TileContext(nc) as tc, ExitStack() as ctx:
        cpool = ctx.enter_context(tc.tile_pool(name="const", bufs=1))
        fpool = ctx.enter_context(tc.tile_pool(name="feats", bufs=3))
        epool = ctx.enter_context(tc.tile_pool(name="ech", bufs=3))
        upoolA = ctx.enter_context(tc.tile_pool(name="uA", bufs=3))
        upoolB = ctx.enter_context(tc.tile_pool(name="uB", bufs=3))
        psA = ctx.enter_context(tc.tile_pool(name="psA", bufs=2, space="PSUM"))
        psB = ctx.enter_context(tc.tile_pool(name="psB", bufs=2, space="PSUM"))

        pak_s = cpool.tile([128, 128], bf16d)
        nc.sync.dma_start(pak_s[:, :], pak_d[:, :])
        ua = upoolA.tile([128, HC], bf16d)
        nc.sync.dma_start(ua[:, :], u0_d[:, 0:HC])
        ub = upoolB.tile([128, HC], bf16d)
        nc.sync.dma_start(ub[:, :], u0_d[:, HC:COLS])

        base = 0
        for ch in CHUNKS:
            fch = fpool.tile([128, ch, COLS], bf16d)
            nc.sync.dma_start(
                fch[:, :, :],
                featsT_d[base:base + ch, :, :, :].rearrange(
                    "s h k c -> (h k) s c"),
            )
            ech = epool.tile([128, ch, COLS], bf16d)
            nc.scalar.activation(
                ech[:, :, :].rearrange("p a b -> p (a b)"),
                fch[:, :, :].rearrange("p a b -> p (a b)"),
                AF.Exp,
            )
            for i in range(ch):
                pa = psA.tile([128, HC], f32)
                nc.tensor.matmul(pa[:, :], pak_s[:, :], ua[:, :],
                                 start=True, stop=True)
                una = upoolA.tile([128, HC], bf16d)
                nc.vector.tensor_tensor(una[:, :], pa[:, :], ech[:, i, 0:HC],
                                        ALU.mult)
                ua = una

                pb = psB.tile([128, HC], f32)
                nc.tensor.matmul(pb[:, :], pak_s[:, :], ub[:, :],
                                 start=True, stop=True)
                unb = upoolB.tile([128, HC], bf16d)
                nc.vector.tensor_tensor(unb[:, :], pb[:, :], ech[:, i, HC:COLS],
                                        ALU.mult)
                ub = unb
            base += ch

        nc.sync.dma_start(uout_d[:, 0:HC], ua[:, :])
        nc.sync.dma_start(uout_d[:, HC:COLS], ub[:, :])

    nc.compile()
    return nc


def _estimate_ln_scale(feats, transitions, start_tag):
    """Mean per-step log growth of the forward recursion, measured on a few
    examples/steps so the folded scale keeps the exp-domain state centered."""
    n_ex, n_st = 8, 64
    A = np.exp(transitions.astype(np.float64))
    score = np.tile(start_tag.astype(np.float64)[None, :], (n_ex, 1))
    f = feats[:n_ex, :n_st, :].astype(np.float64)
    lam0 = lamN = None
    for s in range(n_st):
        m = score.max(1, keepdims=True)
        score = np.log(np.exp(score - m) @ A.T) + m + f[:, s, :]
        lse = np.log(np.exp(score - score.max(1, keepdims=True)).sum(1)) \
            + score.max(1)
        if s == 0:
            lam0 = lse
        lamN = lse
    return -float((lamN - lam0).mean() / (n_st - 1))


def _pack_state(vec):
    """[T, B] per-example state -> [128, COLS]: row h*64+k, col c = ex 512h+c."""
    return np.ascontiguousarray(
        vec.reshape(T, 2, COLS).transpose(1, 0, 2).reshape(128, COLS))


def _unpack_state(arr):
    """[128, COLS] -> [T, B]."""
    return np.asarray(arr).reshape(2, T, COLS).transpose(1, 0, 2).reshape(T, B)


def _host_inputs(feats, transitions, start_tag):
    """Per-core input tensors. Returns (in_maps, ln_scale, z4)."""
    ln_scale = _estimate_ln_scale(feats, transitions, start_tag)
    Ap = np.exp(transitions.astype(np.float64) + ln_scale)

    def pack_blockdiag(m):
        out = np.zeros((128, 128), dtype=np.float32)
        out[:T, :T] = m
        out[T:, T:] = m
        return out.astype(bf16)

    pak_fw = pack_blockdiag(Ap.T.astype(np.float32))  # out[j]=Σ_k A'[j,k]u[k]
    pak_bw = pack_blockdiag(Ap.astype(np.float32))    # out[k]=Σ_j A'[j,k]g[j]

    ones_v = np.ones((T, B), np.float32)
    u0_start = np.tile(np.exp(start_tag.astype(np.float32))[:, None], (1, B))
    R = np.exp(transitions[T - 1, :].astype(np.float32))

    def estep(s):
        return np.exp(feats[:, s, :].astype(np.float32)).T  # [T, B]

    # (pak, u0_vec, stream ids: int step or -1 for zeros)
    runs = [
        (pak_fw, u0_start,              list(range(SEG[0], SEG[1]))),
        (pak_fw, ones_v,                list(range(SEG[1], SEG[2]))),
        (pak_bw, estep(SEG[2] - 1),     list(range(SEG[2] - 2, SEG[1] - 1, -1)) + [-1]),
        (pak_fw, ones_v,                list(range(SEG[2], SEG[3]))),
        (pak_bw, estep(SEG[3] - 1),     list(range(SEG[3] - 2, SEG[2] - 1, -1)) + [-1]),
        (pak_fw, ones_v,                [-1] * 3 + list(range(SEG[3], SEG[4]))),
        (pak_bw, ones_v,                [-1] * 2 + list(range(SEG[4] - 1, SEG[3] - 1, -1)) + [-1]),
        (pak_bw, estep(S - 1) * R[:, None],
         list(range(S - 2, SEG[4] - 1, -1)) + [-1]),
    ]

    fb = np.ascontiguousarray(feats.transpose(1, 2, 0)).astype(bf16)  # [S,T,B]
    zrow = np.zeros((T, B), dtype=bf16)

    in_maps = []
    for pak, u0v, ids in runs:
        assert len(ids) == L, len(ids)
        F = np.empty((L, 2, T, COLS), dtype=bf16)
        for pos, s in enumerate(ids):
            src = fb[s] if s >= 0 else zrow           # [T, B]
            F[pos] = src.reshape(T, 2, COLS).transpose(1, 0, 2)
        in_maps.append({
            "featsT": F,
            "pak": pak,
            "u0": _pack_state(u0v).astype(bf16),
        })

    z4 = (np.linalg.matrix_power(Ap, 3) @ np.ones(T))  # probe seed of run 5
    return in_maps, ln_scale, z4


def _host_gold(feats, transitions, start_tag, tags):
    tags_i = tags.astype(np.int64)
    emit = np.take_along_axis(feats, tags_i[:, :, None], axis=2)[:, :, 0]
    trans_sc = transitions[tags_i[:, :-1], tags_i[:, 1:]]
    gold = (start_tag[tags_i[:, 0]] + emit.sum(1, dtype=np.float64)
            + trans_sc.sum(1, dtype=np.float64) + start_tag[tags_i[:, -1]])
    return gold


def _assemble(results, ln_scale, z4):
    """results: list of 8 {'uout': [128, COLS]} -> fwd [B] (float64)."""
    f1 = _unpack_state(results[0]["uout"]).astype(np.float64)
    f2 = _unpack_state(results[1]["uout"]).astype(np.float64)
    g2 = _unpack_state(results[2]["uout"]).astype(np.float64)
    f3 = _unpack_state(results[3]["uout"]).astype(np.float64)
    g3 = _unpack_state(results[4]["uout"]).astype(np.float64)
    f4 = _unpack_state(results[5]["uout"]).astype(np.float64)
    g4 = _unpack_state(results[6]["uout"]).astype(np.float64)
    b5 = _unpack_state(results[7]["uout"]).astype(np.float64)

    num = (np.log((b5 * f4).sum(0)) + np.log((g4 * f3).sum(0))
           + np.log((g3 * f2).sum(0)) + np.log((g2 * f1).sum(0)))
    den = (np.log((g4 * z4[:, None]).sum(0)) + np.log(g3.sum(0))
           + np.log(g2.sum(0)))
    return num - den - S * ln_scale


_NC_CACHE = {}


def _get_program():
    if "nc" not in _NC_CACHE:
        _NC_CACHE["nc"] = _build_program()
    return _NC_CACHE["nc"]


def kernel(feats, transitions, start_tag, tags, mask_x, len_seq):
    feats = np.asarray(feats, dtype=np.float32)
    transitions = np.asarray(transitions, dtype=np.float32)
    start_tag = np.asarray(start_tag, dtype=np.float32)
    tags_np = np.asarray(tags)

    in_maps, ln_scale, z4 = _host_inputs(feats, transitions, start_tag)
    nc = _get_program()
    res = run_bass_kernel_spmd(nc, in_maps, list(range(NC)))

    fwd = _assemble(res.results, ln_scale, z4)
    gold = _host_gold(feats, transitions, start_tag, tags_np)
    return (fwd - gold).astype(np.float32)


# revision 16
# speedup vs baseline: 8.0933x; 1.0060x over previous
"""CRF log-loss kernel for TRN2 — 5-way sequence split with rank-1 junctions.

The S=512-step forward recursion U_{s+1} = e_s ∘ (A' U_s) is split into 5
segments. Products of positive matrices contract to rank-1 exponentially
fast (measured sigma2/sigma1 ~ 1e-16 over 100 steps here), so each middle
segment's transfer matrix M_i is represented by a forward probe f_i = M_i z
and a backward probe g_i = M_i^T w:  M_i ≈ f_i g_i^T / (w^T M_i z), exact
for rank-1. The full path value

  r^T M5 M4 M3 M2 M1 u0  ≈  (b5·f4)(g4·f3)(g3·f2)(g2·f1) / (c4 c3 c2)

is assembled on the host from per-example 64-vector dots. 8 cores run the
IDENTICAL 103-step program (one [128,128]x[128,512] bf16 matmul + one
[128,512] multiply per step, two independently-semaphored 256-column
chains to hide latency), differing only in inputs: stationary matrix
(A'^T-pack for forward runs, A'-pack for backward), init state, and feats
stream (per-step [tag, example] slices prepared on the host in bf16;
backward streams are time-reversed, closed with a zeros step exp(0)=1 that
realizes the trailing bare A'^T, and short segments are padded with leading
zeros steps that merely warm the arbitrary probe seed). The drift constant
LN_SCALE (estimated from the data at run time) is folded into A', so the
exp-domain state needs no renormalization. The gold path is host-side
index plumbing.
"""
import numpy as np
import ml_dtypes
from contextlib import ExitStack

import concourse.bass as bass
import concourse.bacc as bacc
import concourse.tile as tile
import concourse.mybir as mybir
from concourse.bass_utils import run_bass_kernel_spmd

bf16 = ml_dtypes.bfloat16
f32 = mybir.dt.float32
bf16d = mybir.dt.bfloat16

B, S, T = 1024, 512, 64
NC = 8
L = 103                  # program steps per core
COLS = 512               # matmul columns (1024 examples packed 2-per-column)
HC = COLS // 2           # columns per chain
CHUNKS = [2] + [8] * 12 + [5]  # DMA/exp chunk sizes, sum = 103; small first
                               # chunk so the chain starts sooner

AF = mybir.ActivationFunctionType
ALU = mybir.AluOpType

# segment boundaries [0,103),[103,206),[206,309),[309,409),[409,512)
SEG = [0, 103, 206, 309, 409, 512]


def _build_program():
    nc = bacc.Bacc("TRN2", target_bir_lowering=False, debug=False, num_devices=NC)

    featsT_d = nc.dram_tensor("featsT", [L, 2, T, COLS], bf16d, kind="ExternalInput")
    pak_d = nc.dram_tensor("pak", [128, 128], bf16d, kind="ExternalInput")
    u0_d = nc.dram_tensor("u0", [128, COLS], bf16d, kind="ExternalInput")
    uout_d = nc.dram_tensor("uout", [128, COLS], bf16d, kind="ExternalOutput")

    # chains: (col_lo, col_hi, engine)
    chains = [(0, 256, "vector"), (256, 512, "vector")]

    with tile.TileContext(nc) as tc, ExitStack() as ctx:
        cpool = ctx.enter_context(tc.tile_pool(name="const", bufs=1))
        fpool = ctx.enter_context(tc.tile_pool(name="feats", bufs=3))
        epool = ctx.enter_context(tc.tile_pool(name="ech", bufs=3))
        upools = [ctx.enter_context(tc.tile_pool(name=f"u{i}", bufs=3))
                  for i in range(len(chains))]
        pspools = [ctx.enter_context(
            tc.tile_pool(name=f"ps{i}", bufs=2, space="PSUM"))
            for i in range(len(chains))]

        pak_s = cpool.tile([128, 128], bf16d)
        nc.sync.dma_start(pak_s[:, :], pak_d[:, :])
        us = []
        for ci, (lo, hi, _) in enumerate(chains):
            u = upools[ci].tile([128, hi - lo], bf16d)
            nc.sync.dma_start(u[:, :], u0_d[:, lo:hi])
            us.append(u)

        base = 0
        for ch in CHUNKS:
            fch = fpool.tile([128, ch, COLS], bf16d)
            nc.sync.dma_start(
                fch[:, :, :],
                featsT_d[base:base + ch, :, :, :].rearrange(
                    "s h k c -> (h k) s c"),
            )
            ech = epool.tile([128, ch, COLS], bf16d)
            nc.scalar.activation(
                ech[:, :, :].rearrange("p a b -> p (a b)"),
                fch[:, :, :].rearrange("p a b -> p (a b)"),
                AF.Exp,
            )
            for i in range(ch):
                for ci, (lo, hi, eng) in enumerate(chains):
                    pt = pspools[ci].tile([128, hi - lo], f32)
                    nc.tensor.matmul(pt[:, :], pak_s[:, :], us[ci][:, :],
                                     start=True, stop=True)
                    un = upools[ci].tile([128, hi - lo], bf16d)
                    getattr(nc, eng).tensor_tensor(
                        un[:, :], pt[:, :], ech[:, i, lo:hi], ALU.mult)
                    us[ci] = un
            base += ch

        for ci, (lo, hi, _) in enumerate(chains):
            nc.sync.dma_start(uout_d[:, lo:hi], us[ci][:, :])

    nc.compile()
    return nc


def _estimate_ln_scale(feats, transitions, start_tag):
    """Mean per-step log growth of the forward recursion, measured on a few
    examples/steps so the folded scale keeps the exp-domain state centered."""
    n_ex, n_st = 8, 64
    A = np.exp(transitions.astype(np.float64))
    score = np.tile(start_tag.astype(np.float64)[None, :], (n_ex, 1))
    f = feats[:n_ex, :n_st, :].astype(np.float64)
    lam0 = lamN = None
    for s in range(n_st):
        m = score.max(1, keepdims=True)
        score = np.log(np.exp(score - m) @ A.T) + m + f[:, s, :]
        lse = np.log(np.exp(score - score.max(1, keepdims=True)).sum(1)) \
            + score.max(1)
        if s == 0:
            lam0 = lse
        lamN = lse
    return -float((lamN - lam0).mean() / (n_st - 1))


def _pack_state(vec):
    """[T, B] per-example state -> [128, COLS]: row h*64+k, col c = ex 512h+c."""
    return np.ascontiguousarray(
        vec.reshape(T, 2, COLS).transpose(1, 0, 2).reshape(128, COLS))


def _unpack_state(arr):
    """[128, COLS] -> [T, B]."""
    return np.asarray(arr).reshape(2, T, COLS).transpose(1, 0, 2).reshape(T, B)


def _host_inputs(feats, transitions, start_tag):
    """Per-core input tensors. Returns (in_maps, ln_scale, z4)."""
    ln_scale = _estimate_ln_scale(feats, transitions, start_tag)
    Ap = np.exp(transitions.astype(np.float64) + ln_scale)

    def pack_blockdiag(m):
        out = np.zeros((128, 128), dtype=np.float32)
        out[:T, :T] = m
        out[T:, T:] = m
        return out.astype(bf16)

    pak_fw = pack_blockdiag(Ap.T.astype(np.float32))  # out[j]=Σ_k A'[j,k]u[k]
    pak_bw = pack_blockdiag(Ap.astype(np.float32))    # out[k]=Σ_j A'[j,k]g[j]

    ones_v = np.ones((T, B), np.float32)
    u0_start = np.tile(np.exp(start_tag.astype(np.float32))[:, None], (1, B))
    R = np.exp(transitions[T - 1, :].astype(np.float32))

    def estep(s):
        return np.exp(feats[:, s, :].astype(np.float32)).T  # [T, B]

    # (pak, u0_vec, stream ids: int step or -1 for zeros)
    runs = [
        (pak_fw, u0_start,              list(range(SEG[0], SEG[1]))),
        (pak_fw, ones_v,                list(range(SEG[1], SEG[2]))),
        (pak_bw, estep(SEG[2] - 1),     list(range(SEG[2] - 2, SEG[1] - 1, -1)) + [-1]),
        (pak_fw, ones_v,                list(range(SEG[2], SEG[3]))),
        (pak_bw, estep(SEG[3] - 1),     list(range(SEG[3] - 2, SEG[2] - 1, -1)) + [-1]),
        (pak_fw, ones_v,                [-1] * 3 + list(range(SEG[3], SEG[4]))),
        (pak_bw, ones_v,                [-1] * 2 + list(range(SEG[4] - 1, SEG[3] - 1, -1)) + [-1]),
        (pak_bw, estep(S - 1) * R[:, None],
         list(range(S - 2, SEG[4] - 1, -1)) + [-1]),
    ]

    fb = np.ascontiguousarray(feats.transpose(1, 2, 0)).astype(bf16)  # [S,T,B]
    zrow = np.zeros((T, B), dtype=bf16)

    in_maps = []
    for pak, u0v, ids in runs:
        assert len(ids) == L, len(ids)
        F = np.empty((L, 2, T, COLS), dtype=bf16)
        for pos, s in enumerate(ids):
            src = fb[s] if s >= 0 else zrow           # [T, B]
            F[pos] = src.reshape(T, 2, COLS).transpose(1, 0, 2)
        in_maps.append({
            "featsT": F,
            "pak": pak,
            "u0": _pack_state(u0v).astype(bf16),
        })

    z4 = (np.linalg.matrix_power(Ap, 3) @ np.ones(T))  # probe seed of run 5
    return in_maps, ln_scale, z4


def _host_gold(feats, transitions, start_tag, tags):
    tags_i = tags.astype(np.int64)
    emit = np.take_along_axis(feats, tags_i[:, :, None], axis=2)[:, :, 0]
    trans_sc = transitions[tags_i[:, :-1], tags_i[:, 1:]]
    gold = (start_tag[tags_i[:, 0]] + emit.sum(1, dtype=np.float64)
            + trans_sc.sum(1, dtype=np.float64) + start_tag[tags_i[:, -1]])
    return gold


def _assemble(results, ln_scale, z4):
    """results: list of 8 {'uout': [128, COLS]} -> fwd [B] (float64)."""
    f1 = _unpack_state(results[0]["uout"]).astype(np.float64)
    f2 = _unpack_state(results[1]["uout"]).astype(np.float64)
    g2 = _unpack_state(results[2]["uout"]).astype(np.float64)
    f3 = _unpack_state(results[3]["uout"]).astype(np.float64)
    g3 = _unpack_state(results[4]["uout"]).astype(np.float64)
    f4 = _unpack_state(results[5]["uout"]).astype(np.float64)
    g4 = _unpack_state(results[6]["uout"]).astype(np.float64)
    b5 = _unpack_state(results[7]["uout"]).astype(np.float64)

    num = (np.log((b5 * f4).sum(0)) + np.log((g4 * f3).sum(0))
           + np.log((g3 * f2).sum(0)) + np.log((g2 * f1).sum(0)))
    den = (np.log((g4 * z4[:, None]).sum(0)) + np.log(g3.sum(0))
           + np.log(g2.sum(0)))
    return num - den - S * ln_scale


_NC_CACHE = {}


def _get_program():
    if "nc" not in _NC_CACHE:
        _NC_CACHE["nc"] = _build_program()
    return _NC_CACHE["nc"]


def kernel(feats, transitions, start_tag, tags, mask_x, len_seq):
    feats = np.asarray(feats, dtype=np.float32)
    transitions = np.asarray(transitions, dtype=np.float32)
    start_tag = np.asarray(start_tag, dtype=np.float32)
    tags_np = np.asarray(tags)

    in_maps, ln_scale, z4 = _host_inputs(feats, transitions, start_tag)
    nc = _get_program()
    res = run_bass_kernel_spmd(nc, in_maps, list(range(NC)))

    fwd = _assemble(res.results, ln_scale, z4)
    gold = _host_gold(feats, transitions, start_tag, tags_np)
    return (fwd - gold).astype(np.float32)


# revision 17
# speedup vs baseline: 8.1312x; 1.0047x over previous
"""CRF log-loss kernel for TRN2 — 5-way sequence split with rank-1 junctions.

The S=512-step forward recursion U_{s+1} = e_s ∘ (A' U_s) is split into 5
segments. Products of positive matrices contract to rank-1 exponentially
fast (measured sigma2/sigma1 ~ 1e-16 over 100 steps here), so each middle
segment's transfer matrix M_i is represented by a forward probe f_i = M_i z
and a backward probe g_i = M_i^T w:  M_i ≈ f_i g_i^T / (w^T M_i z), exact
for rank-1. The full path value

  r^T M5 M4 M3 M2 M1 u0  ≈  (b5·f4)(g4·f3)(g3·f2)(g2·f1) / (c4 c3 c2)

is assembled on the host from per-example 64-vector dots. 8 cores run the
IDENTICAL 103-step program (one [128,128]x[128,512] bf16 matmul + one
[128,512] multiply per step, two independently-semaphored 256-column
chains to hide latency), differing only in inputs: stationary matrix
(A'^T-pack for forward runs, A'-pack for backward), init state, and feats
stream (per-step [tag, example] slices prepared on the host in bf16;
backward streams are time-reversed, closed with a zeros step exp(0)=1 that
realizes the trailing bare A'^T, and short segments are padded with leading
zeros steps that merely warm the arbitrary probe seed). The drift constant
LN_SCALE (estimated from the data at run time) is folded into A', so the
exp-domain state needs no renormalization. The gold path is host-side
index plumbing.
"""
import numpy as np
import ml_dtypes
from contextlib import ExitStack

import concourse.bass as bass
import concourse.bacc as bacc
import concourse.tile as tile
import concourse.mybir as mybir
from concourse.bass_utils import run_bass_kernel_spmd

bf16 = ml_dtypes.bfloat16
f32 = mybir.dt.float32
bf16d = mybir.dt.bfloat16

B, S, T = 1024, 512, 64
NC = 8
L = 103                  # program steps per core
COLS = 512               # matmul columns (1024 examples packed 2-per-column)
HC = COLS // 2           # columns per chain
CHUNKS = [2] + [8] * 12 + [5]  # DMA/exp chunk sizes, sum = 103; small first
                               # chunk so the chain starts sooner

AF = mybir.ActivationFunctionType
ALU = mybir.AluOpType

# segment boundaries [0,103),[103,206),[206,309),[309,409),[409,512)
SEG = [0, 103, 206, 309, 409, 512]


def _build_program():
    nc = bacc.Bacc("TRN2", target_bir_lowering=False, debug=False, num_devices=NC)

    featsT_d = nc.dram_tensor("featsT", [L, 2, T, COLS], bf16d, kind="ExternalInput")
    pak_d = nc.dram_tensor("pak", [128, 128], bf16d, kind="ExternalInput")
    u0_d = nc.dram_tensor("u0", [128, COLS], bf16d, kind="ExternalInput")
    uout_d = nc.dram_tensor("uout", [128, COLS], bf16d, kind="ExternalOutput")

    # chains: (col_lo, col_hi, engine)
    chains = [(0, 256, "vector"), (256, 512, "vector")]

    with tile.TileContext(nc) as tc, ExitStack() as ctx:
        cpool = ctx.enter_context(tc.tile_pool(name="const", bufs=1))
        fpool = ctx.enter_context(tc.tile_pool(name="feats", bufs=3))
        epool = ctx.enter_context(tc.tile_pool(name="ech", bufs=3))
        upools = [ctx.enter_context(tc.tile_pool(name=f"u{i}", bufs=3))
                  for i in range(len(chains))]
        pspools = [ctx.enter_context(
            tc.tile_pool(name=f"ps{i}", bufs=2, space="PSUM"))
            for i in range(len(chains))]

        # head DMAs on the otherwise-idle gpsimd SWDGE queue, overlapping the
        # first feats-chunk DMA on the sync queue and leaving the scalar
        # queue free for its activation-table load
        pak_s = cpool.tile([128, 128], bf16d)
        nc.gpsimd.dma_start(pak_s[:, :], pak_d[:, :])
        us = []
        for ci, (lo, hi, _) in enumerate(chains):
            u = upools[ci].tile([128, hi - lo], bf16d)
            nc.gpsimd.dma_start(u[:, :], u0_d[:, lo:hi])
            us.append(u)

        base = 0
        for ch in CHUNKS:
            fch = fpool.tile([128, ch, COLS], bf16d)
            nc.sync.dma_start(
                fch[:, :, :],
                featsT_d[base:base + ch, :, :, :].rearrange(
                    "s h k c -> (h k) s c"),
            )
            ech = epool.tile([128, ch, COLS], bf16d)
            nc.scalar.activation(
                ech[:, :, :].rearrange("p a b -> p (a b)"),
                fch[:, :, :].rearrange("p a b -> p (a b)"),
                AF.Exp,
            )
            for i in range(ch):
                for ci, (lo, hi, eng) in enumerate(chains):
                    pt = pspools[ci].tile([128, hi - lo], f32)
                    nc.tensor.matmul(pt[:, :], pak_s[:, :], us[ci][:, :],
                                     start=True, stop=True)
                    un = upools[ci].tile([128, hi - lo], bf16d)
                    getattr(nc, eng).tensor_tensor(
                        un[:, :], pt[:, :], ech[:, i, lo:hi], ALU.mult)
                    us[ci] = un
            base += ch

        for ci, (lo, hi, _) in enumerate(chains):
            nc.sync.dma_start(uout_d[:, lo:hi], us[ci][:, :])

    nc.compile()
    return nc


def _estimate_ln_scale(feats, transitions, start_tag):
    """Mean per-step log growth of the forward recursion, measured on a few
    examples/steps so the folded scale keeps the exp-domain state centered."""
    n_ex, n_st = 8, 64
    A = np.exp(transitions.astype(np.float64))
    score = np.tile(start_tag.astype(np.float64)[None, :], (n_ex, 1))
    f = feats[:n_ex, :n_st, :].astype(np.float64)
    lam0 = lamN = None
    for s in range(n_st):
        m = score.max(1, keepdims=True)
        score = np.log(np.exp(score - m) @ A.T) + m + f[:, s, :]
        lse = np.log(np.exp(score - score.max(1, keepdims=True)).sum(1)) \
            + score.max(1)
        if s == 0:
            lam0 = lse
        lamN = lse
    return -float((lamN - lam0).mean() / (n_st - 1))


def _pack_state(vec):
    """[T, B] per-example state -> [128, COLS]: row h*64+k, col c = ex 512h+c."""
    return np.ascontiguousarray(
        vec.reshape(T, 2, COLS).transpose(1, 0, 2).reshape(128, COLS))


def _unpack_state(arr):
    """[128, COLS] -> [T, B]."""
    return np.asarray(arr).reshape(2, T, COLS).transpose(1, 0, 2).reshape(T, B)


def _host_inputs(feats, transitions, start_tag):
    """Per-core input tensors. Returns (in_maps, ln_scale, z4)."""
    ln_scale = _estimate_ln_scale(feats, transitions, start_tag)
    Ap = np.exp(transitions.astype(np.float64) + ln_scale)

    def pack_blockdiag(m):
        out = np.zeros((128, 128), dtype=np.float32)
        out[:T, :T] = m
        out[T:, T:] = m
        return out.astype(bf16)

    pak_fw = pack_blockdiag(Ap.T.astype(np.float32))  # out[j]=Σ_k A'[j,k]u[k]
    pak_bw = pack_blockdiag(Ap.astype(np.float32))    # out[k]=Σ_j A'[j,k]g[j]

    ones_v = np.ones((T, B), np.float32)
    u0_start = np.tile(np.exp(start_tag.astype(np.float32))[:, None], (1, B))
    R = np.exp(transitions[T - 1, :].astype(np.float32))

    def estep(s):
        return np.exp(feats[:, s, :].astype(np.float32)).T  # [T, B]

    # (pak, u0_vec, stream ids: int step or -1 for zeros)
    runs = [
        (pak_fw, u0_start,              list(range(SEG[0], SEG[1]))),
        (pak_fw, ones_v,                list(range(SEG[1], SEG[2]))),
        (pak_bw, estep(SEG[2] - 1),     list(range(SEG[2] - 2, SEG[1] - 1, -1)) + [-1]),
        (pak_fw, ones_v,                list(range(SEG[2], SEG[3]))),
        (pak_bw, estep(SEG[3] - 1),     list(range(SEG[3] - 2, SEG[2] - 1, -1)) + [-1]),
        (pak_fw, ones_v,                [-1] * 3 + list(range(SEG[3], SEG[4]))),
        (pak_bw, ones_v,                [-1] * 2 + list(range(SEG[4] - 1, SEG[3] - 1, -1)) + [-1]),
        (pak_bw, estep(S - 1) * R[:, None],
         list(range(S - 2, SEG[4] - 1, -1)) + [-1]),
    ]

    fb = np.ascontiguousarray(feats.transpose(1, 2, 0)).astype(bf16)  # [S,T,B]
    zrow = np.zeros((T, B), dtype=bf16)

    in_maps = []
    for pak, u0v, ids in runs:
        assert len(ids) == L, len(ids)
        F = np.empty((L, 2, T, COLS), dtype=bf16)
        for pos, s in enumerate(ids):
            src = fb[s] if s >= 0 else zrow           # [T, B]
            F[pos] = src.reshape(T, 2, COLS).transpose(1, 0, 2)
        in_maps.append({
            "featsT": F,
            "pak": pak,
            "u0": _pack_state(u0v).astype(bf16),
        })

    z4 = (np.linalg.matrix_power(Ap, 3) @ np.ones(T))  # probe seed of run 5
    return in_maps, ln_scale, z4


def _host_gold(feats, transitions, start_tag, tags):
    tags_i = tags.astype(np.int64)
    emit = np.take_along_axis(feats, tags_i[:, :, None], axis=2)[:, :, 0]
    trans_sc = transitions[tags_i[:, :-1], tags_i[:, 1:]]
    gold = (start_tag[tags_i[:, 0]] + emit.sum(1, dtype=np.float64)
            + trans_sc.sum(1, dtype=np.float64) + start_tag[tags_i[:, -1]])
    return gold


def _assemble(results, ln_scale, z4):
    """results: list of 8 {'uout': [128, COLS]} -> fwd [B] (float64)."""
    f1 = _unpack_state(results[0]["uout"]).astype(np.float64)
    f2 = _unpack_state(results[1]["uout"]).astype(np.float64)
    g2 = _unpack_state(results[2]["uout"]).astype(np.float64)
    f3 = _unpack_state(results[3]["uout"]).astype(np.float64)
    g3 = _unpack_state(results[4]["uout"]).astype(np.float64)
    f4 = _unpack_state(results[5]["uout"]).astype(np.float64)
    g4 = _unpack_state(results[6]["uout"]).astype(np.float64)
    b5 = _unpack_state(results[7]["uout"]).astype(np.float64)

    num = (np.log((b5 * f4).sum(0)) + np.log((g4 * f3).sum(0))
           + np.log((g3 * f2).sum(0)) + np.log((g2 * f1).sum(0)))
    den = (np.log((g4 * z4[:, None]).sum(0)) + np.log(g3.sum(0))
           + np.log(g2.sum(0)))
    return num - den - S * ln_scale


_NC_CACHE = {}


def _get_program():
    if "nc" not in _NC_CACHE:
        _NC_CACHE["nc"] = _build_program()
    return _NC_CACHE["nc"]


def kernel(feats, transitions, start_tag, tags, mask_x, len_seq):
    feats = np.asarray(feats, dtype=np.float32)
    transitions = np.asarray(transitions, dtype=np.float32)
    start_tag = np.asarray(start_tag, dtype=np.float32)
    tags_np = np.asarray(tags)

    in_maps, ln_scale, z4 = _host_inputs(feats, transitions, start_tag)
    nc = _get_program()
    res = run_bass_kernel_spmd(nc, in_maps, list(range(NC)))

    fwd = _assemble(res.results, ln_scale, z4)
    gold = _host_gold(feats, transitions, start_tag, tags_np)
    return (fwd - gold).astype(np.float32)


# revision 21
# speedup vs baseline: 8.3735x; 1.0298x over previous
"""CRF log-loss kernel for TRN2 — 5-way sequence split with rank-1 junctions.

The S=512-step forward recursion U_{s+1} = e_s ∘ (A' U_s) is split into 5
segments. Products of positive matrices contract to rank-1 exponentially
fast (measured sigma2/sigma1 ~ 1e-16 over 100 steps here), so each middle
segment's transfer matrix M_i is represented by a forward probe f_i = M_i z
and a backward probe g_i = M_i^T w:  M_i ≈ f_i g_i^T / (w^T M_i z), exact
for rank-1. The full path value

  r^T M5 M4 M3 M2 M1 u0  ≈  (b5·f4)(g4·f3)(g3·f2)(g2·f1) / (c4 c3 c2)

is assembled on the host from per-example 64-vector dots. 8 cores run the
IDENTICAL 103-step program (one [128,128]x[128,512] bf16 matmul + one
[128,512] multiply per step, two independently-semaphored 256-column
chains to hide latency), differing only in inputs: stationary matrix
(A'^T-pack for forward runs, A'-pack for backward), init state, and feats
stream (per-step [tag, example] slices prepared on the host in bf16;
backward streams are time-reversed, closed with a zeros step exp(0)=1 that
realizes the trailing bare A'^T, and short segments are padded with leading
zeros steps that merely warm the arbitrary probe seed). The drift constant
LN_SCALE (estimated from the data at run time) is folded into A', so the
exp-domain state needs no renormalization. The gold path is host-side
index plumbing.
"""
import numpy as np
import ml_dtypes
from contextlib import ExitStack

import concourse.bass as bass
import concourse.bacc as bacc
import concourse.tile as tile
import concourse.mybir as mybir
from concourse.bass_utils import run_bass_kernel_spmd

bf16 = ml_dtypes.bfloat16
f32 = mybir.dt.float32
bf16d = mybir.dt.bfloat16

B, S, T = 1024, 512, 64
NC = 8
L = 103                  # program steps per core
COLS = 512               # matmul columns (1024 examples packed 2-per-column)
HC = COLS // 2           # columns per chain
CHUNKS = [3, 4, 6, 7] + [8] * 10 + [3]  # DMA/exp chunk sizes, sum = 103;
                            # graduated ramp so the chain never outruns the
                            # serialized DMA-transfer -> exp head pipeline

AF = mybir.ActivationFunctionType
ALU = mybir.AluOpType

# segment boundaries [0,103),[103,206),[206,309),[309,409),[409,512)
SEG = [0, 103, 206, 309, 409, 512]


def _build_program():
    nc = bacc.Bacc("TRN2", target_bir_lowering=False, debug=False, num_devices=NC)

    featsT_d = nc.dram_tensor("featsT", [L, 2, T, COLS], bf16d, kind="ExternalInput")
    pak_d = nc.dram_tensor("pak", [128, 128], bf16d, kind="ExternalInput")
    u0_d = nc.dram_tensor("u0", [128, COLS], bf16d, kind="ExternalInput")
    uout_d = nc.dram_tensor("uout", [128, COLS], bf16d, kind="ExternalOutput")

    # chains: (col_lo, col_hi, engine)
    chains = [(0, 256, "vector"), (256, 512, "vector")]

    with tile.TileContext(nc) as tc, ExitStack() as ctx:
        cpool = ctx.enter_context(tc.tile_pool(name="const", bufs=1))
        fpool = ctx.enter_context(tc.tile_pool(name="feats", bufs=3))
        epool = ctx.enter_context(tc.tile_pool(name="ech", bufs=3))
        upools = [ctx.enter_context(tc.tile_pool(name=f"u{i}", bufs=3))
                  for i in range(len(chains))]
        pspools = [ctx.enter_context(
            tc.tile_pool(name=f"ps{i}", bufs=2, space="PSUM"))
            for i in range(len(chains))]

        def load_chunk(base, ch):
            fch = fpool.tile([128, ch, COLS], bf16d)
            nc.sync.dma_start(
                fch[:, :, :],
                featsT_d[base:base + ch, :, :, :].rearrange(
                    "s h k c -> (h k) s c"),
            )
            ech = epool.tile([128, ch, COLS], bf16d)
            nc.scalar.activation(
                ech[:, :, :].rearrange("p a b -> p (a b)"),
                fch[:, :, :].rearrange("p a b -> p (a b)"),
                AF.Exp,
            )
            return ech

        # first feats chunk ahead of pak/u0 on the sync queue: the chain's
        # first multiply gates on exp(chunk 0), not on pak/u0
        ech0 = load_chunk(0, CHUNKS[0])

        pak_s = cpool.tile([128, 128], bf16d)
        nc.sync.dma_start(pak_s[:, :], pak_d[:, :])
        us = []
        for ci, (lo, hi, _) in enumerate(chains):
            u = upools[ci].tile([128, hi - lo], bf16d)
            nc.sync.dma_start(u[:, :], u0_d[:, lo:hi])
            us.append(u)

        base = 0
        for nch, ch in enumerate(CHUNKS):
            ech = ech0 if nch == 0 else load_chunk(base, ch)
            for i in range(ch):
                for ci, (lo, hi, eng) in enumerate(chains):
                    pt = pspools[ci].tile([128, hi - lo], f32)
                    nc.tensor.matmul(pt[:, :], pak_s[:, :], us[ci][:, :],
                                     start=True, stop=True)
                    un = upools[ci].tile([128, hi - lo], bf16d)
                    getattr(nc, eng).tensor_tensor(
                        un[:, :], pt[:, :], ech[:, i, lo:hi], ALU.mult)
                    us[ci] = un
            base += ch

        for ci, (lo, hi, _) in enumerate(chains):
            nc.sync.dma_start(uout_d[:, lo:hi], us[ci][:, :])

    nc.compile()
    return nc


def _estimate_ln_scale(feats, transitions, start_tag):
    """Mean per-step log growth of the forward recursion, measured on a few
    examples/steps so the folded scale keeps the exp-domain state centered."""
    n_ex, n_st = 8, 64
    A = np.exp(transitions.astype(np.float64))
    score = np.tile(start_tag.astype(np.float64)[None, :], (n_ex, 1))
    f = feats[:n_ex, :n_st, :].astype(np.float64)
    lam0 = lamN = None
    for s in range(n_st):
        m = score.max(1, keepdims=True)
        score = np.log(np.exp(score - m) @ A.T) + m + f[:, s, :]
        lse = np.log(np.exp(score - score.max(1, keepdims=True)).sum(1)) \
            + score.max(1)
        if s == 0:
            lam0 = lse
        lamN = lse
    return -float((lamN - lam0).mean() / (n_st - 1))


def _pack_state(vec):
    """[T, B] per-example state -> [128, COLS]: row h*64+k, col c = ex 512h+c."""
    return np.ascontiguousarray(
        vec.reshape(T, 2, COLS).transpose(1, 0, 2).reshape(128, COLS))


def _unpack_state(arr):
    """[128, COLS] -> [T, B]."""
    return np.asarray(arr).reshape(2, T, COLS).transpose(1, 0, 2).reshape(T, B)


def _host_inputs(feats, transitions, start_tag):
    """Per-core input tensors. Returns (in_maps, ln_scale, z4)."""
    ln_scale = _estimate_ln_scale(feats, transitions, start_tag)
    Ap = np.exp(transitions.astype(np.float64) + ln_scale)

    def pack_blockdiag(m):
        out = np.zeros((128, 128), dtype=np.float32)
        out[:T, :T] = m
        out[T:, T:] = m
        return out.astype(bf16)

    pak_fw = pack_blockdiag(Ap.T.astype(np.float32))  # out[j]=Σ_k A'[j,k]u[k]
    pak_bw = pack_blockdiag(Ap.astype(np.float32))    # out[k]=Σ_j A'[j,k]g[j]

    ones_v = np.ones((T, B), np.float32)
    u0_start = np.tile(np.exp(start_tag.astype(np.float32))[:, None], (1, B))
    R = np.exp(transitions[T - 1, :].astype(np.float32))

    def estep(s):
        return np.exp(feats[:, s, :].astype(np.float32)).T  # [T, B]

    # (pak, u0_vec, stream ids: int step or -1 for zeros)
    runs = [
        (pak_fw, u0_start,              list(range(SEG[0], SEG[1]))),
        (pak_fw, ones_v,                list(range(SEG[1], SEG[2]))),
        (pak_bw, estep(SEG[2] - 1),     list(range(SEG[2] - 2, SEG[1] - 1, -1)) + [-1]),
        (pak_fw, ones_v,                list(range(SEG[2], SEG[3]))),
        (pak_bw, estep(SEG[3] - 1),     list(range(SEG[3] - 2, SEG[2] - 1, -1)) + [-1]),
        (pak_fw, ones_v,                [-1] * 3 + list(range(SEG[3], SEG[4]))),
        (pak_bw, ones_v,                [-1] * 2 + list(range(SEG[4] - 1, SEG[3] - 1, -1)) + [-1]),
        (pak_bw, estep(S - 1) * R[:, None],
         list(range(S - 2, SEG[4] - 1, -1)) + [-1]),
    ]

    fb = np.ascontiguousarray(feats.transpose(1, 2, 0)).astype(bf16)  # [S,T,B]
    zrow = np.zeros((T, B), dtype=bf16)

    in_maps = []
    for pak, u0v, ids in runs:
        assert len(ids) == L, len(ids)
        F = np.empty((L, 2, T, COLS), dtype=bf16)
        for pos, s in enumerate(ids):
            src = fb[s] if s >= 0 else zrow           # [T, B]
            F[pos] = src.reshape(T, 2, COLS).transpose(1, 0, 2)
        in_maps.append({
            "featsT": F,
            "pak": pak,
            "u0": _pack_state(u0v).astype(bf16),
        })

    z4 = (np.linalg.matrix_power(Ap, 3) @ np.ones(T))  # probe seed of run 5
    return in_maps, ln_scale, z4


def _host_gold(feats, transitions, start_tag, tags):
    tags_i = tags.astype(np.int64)
    emit = np.take_along_axis(feats, tags_i[:, :, None], axis=2)[:, :, 0]
    trans_sc = transitions[tags_i[:, :-1], tags_i[:, 1:]]
    gold = (start_tag[tags_i[:, 0]] + emit.sum(1, dtype=np.float64)
            + trans_sc.sum(1, dtype=np.float64) + start_tag[tags_i[:, -1]])
    return gold


def _assemble(results, ln_scale, z4):
    """results: list of 8 {'uout': [128, COLS]} -> fwd [B] (float64)."""
    f1 = _unpack_state(results[0]["uout"]).astype(np.float64)
    f2 = _unpack_state(results[1]["uout"]).astype(np.float64)
    g2 = _unpack_state(results[2]["uout"]).astype(np.float64)
    f3 = _unpack_state(results[3]["uout"]).astype(np.float64)
    g3 = _unpack_state(results[4]["uout"]).astype(np.float64)
    f4 = _unpack_state(results[5]["uout"]).astype(np.float64)
    g4 = _unpack_state(results[6]["uout"]).astype(np.float64)
    b5 = _unpack_state(results[7]["uout"]).astype(np.float64)

    num = (np.log((b5 * f4).sum(0)) + np.log((g4 * f3).sum(0))
           + np.log((g3 * f2).sum(0)) + np.log((g2 * f1).sum(0)))
    den = (np.log((g4 * z4[:, None]).sum(0)) + np.log(g3.sum(0))
           + np.log(g2.sum(0)))
    return num - den - S * ln_scale


_NC_CACHE = {}


def _get_program():
    if "nc" not in _NC_CACHE:
        _NC_CACHE["nc"] = _build_program()
    return _NC_CACHE["nc"]


def kernel(feats, transitions, start_tag, tags, mask_x, len_seq):
    feats = np.asarray(feats, dtype=np.float32)
    transitions = np.asarray(transitions, dtype=np.float32)
    start_tag = np.asarray(start_tag, dtype=np.float32)
    tags_np = np.asarray(tags)

    in_maps, ln_scale, z4 = _host_inputs(feats, transitions, start_tag)
    nc = _get_program()
    res = run_bass_kernel_spmd(nc, in_maps, list(range(NC)))

    fwd = _assemble(res.results, ln_scale, z4)
    gold = _host_gold(feats, transitions, start_tag, tags_np)
    return (fwd - gold).astype(np.float32)
